# revision 15
# baseline (speedup 1.0000x reference)
"""Trainium2 Bass kernel for a dense transformer block (pre-LN, causal MHA + 4x MLP).

v3: collective-friendly queues + 256-wide dual-head attention strips.

Sharding over 8 NeuronCores: attention is head-sharded 8 ways (each core does
H/8=2 heads for BOTH batches), then AllToAll re-shards activations to
(batch, token-block) shards so the out-projection and MLP run row-sharded with
full weights.

Key structure vs v2:
- The gpsimd (Pool) queue carries ONLY init DMAs + the 3 collectives + late
  wf2 streaming: per-quad affine_select -> precomputed mask tiles * DVE mult;
  partition_broadcast -> PE outer-product (ones x row) into PSUM.  A blocked
  collective no longer stalls attention (v2 lost ~50us to this).
- Attention runs 256-wide q strips; QK does both heads CONCURRENTLY via PE
  row-tiling (lhsT base partitions 0/64 -> disjoint row groups), halving QK.
  AV runs N=256 fill-bound.  Causal masking multiplies the two diagonal
  chunks by constant [128,256] 0/1 masks on DVE.
- V reaches token-major v_sb via DMA transpose (2 per 128-token chunk per
  head), removing PE transposes + PSUM->SBUF copies.
- LN stats use Rsqrt (one ACT op) instead of Ln/Exp chains.
- The out-projection residual reuses the bf16 x own-chunk already in SBUF
  (v2 loaded a separate 2MB f32 copy).
- Phase order after attention: oproj/LN2/fc1 run per 256-col piece so fc1 of
  piece 0 covers a2a(1); wf1 streams on the scalar queue between exp bursts,
  wf2 on the Pool queue after a2a(1).
"""
import numpy as np
from contextlib import ExitStack

import concourse.bass as bass
import concourse.mybir as mybir
import concourse.tile as tile
from concourse import bacc

F32 = mybir.dt.float32
BF16 = mybir.dt.bfloat16
AF = mybir.ActivationFunctionType
ALU = mybir.AluOpType


class Cfg:
    def __init__(self, D=1024, DFF=4096, H=16, T=2048, B=2, TP=4, HD=64):
        self.D, self.DFF, self.H, self.T, self.B, self.TP, self.HD = D, DFF, H, T, B, TP, HD
        self.NC = B * TP              # cores
        self.KD = D // 128            # feature chunks
        self.FD = DFF // 128          # hidden chunks
        self.LCH = 512                # token chunk (stats shard + stream)
        self.NCH = B * T // self.LCH  # flat chunks (== NC)
        self.NTL = T // self.LCH      # chunks per batch
        self.NHC = H // self.NC       # heads per core
        self.HC = self.NHC * HD       # head feature columns per core
        self.TQ = T // TP             # owned tokens per core
        self.NKC = T // 128           # key chunks per batch
        self.SW = 256                 # q strip width
        self.NST = T // self.SW       # strips per batch
        self.QB = T // 512            # q blocks per batch
        self.HCB = 512                # fc1 weight streaming column block
        self.KHB = 8                  # fc2 weight streaming k block
        self.OCB = 256                # fc2 out column block
        assert self.NCH == self.NC and self.HC == 128


CFG = Cfg()


def emit(ctx: ExitStack, tc: tile.TileContext, io: dict, cfg: Cfg):
    nc = tc.nc
    c = cfg
    rearr = lambda ap: ap.rearrange("(o p) t -> p o t", p=128)

    def mm(ps, lhsT, rhs, start, stop):
        nc.tensor.matmul(ps, lhsT, rhs, start=start, stop=stop)

    # ---------------- constant / persistent pools ----------------
    const = ctx.enter_context(tc.tile_pool(name="const", bufs=1))
    small = ctx.enter_context(tc.tile_pool(name="small", bufs=1))
    dram = ctx.enter_context(tc.tile_pool(name="dram", bufs=1, space="DRAM"))
    wgt = ctx.enter_context(tc.tile_pool(name="wgt", bufs=1))

    from concourse.masks import make_identity
    ident = const.tile([128, 128], BF16, tag="ident")
    make_identity(nc, ident[:])
    ones = const.tile([128, 1], BF16, tag="ones")        # stats lhsT
    nc.gpsimd.memset(ones[:], 1.0)
    ones_c = const.tile([1, 128], BF16, tag="ones_c")    # bcast outer lhsT
    nc.gpsimd.memset(ones_c[:], 1.0)
    ones_f = const.tile([128, 1], F32, tag="ones_f")
    nc.gpsimd.memset(ones_f[:], 1.0)
    eps_t = const.tile([1, 1], F32, tag="eps")
    nc.gpsimd.memset(eps_t[:], 1e-5)
    # causal masks for the two diagonal 128x256 chunk alignments
    msrc = const.tile([128, 256], BF16, tag="msrc")
    nc.gpsimd.memset(msrc[:], 1.0)
    M0 = const.tile([128, 256], BF16, tag="M0")
    nc.gpsimd.affine_select(out=M0[:], in_=msrc[:], compare_op=ALU.is_ge,
                            fill=0.0, base=0, pattern=[[1, 256]],
                            channel_multiplier=-1)
    M1 = const.tile([128, 256], BF16, tag="M1")
    nc.gpsimd.affine_select(out=M1[:], in_=msrc[:], compare_op=ALU.is_ge,
                            fill=0.0, base=-128, pattern=[[1, 256]],
                            channel_multiplier=-1)

    # small parameter tiles (fp32 biases packed per-feature)
    pm = const.tile([128, 2 * c.KD + c.FD], F32, tag="pm", name="pm")
    nc.gpsimd.dma_start(pm[:], io["pm"][:])
    bo = pm[:, 0:c.KD]
    bf1 = pm[:, c.KD:c.KD + c.FD]
    bf2 = pm[:, c.KD + c.FD:]
    # rank-2 LN fixups: rows [wsum; bias] per projection, bf16
    pv2 = const.tile([2, 3 * c.HC], BF16, tag="pv2", name="pv2")
    nc.gpsimd.dma_start(pv2[:], io["pv2"][:])
    pvq, pvk, pvv = (pv2[:, i * c.HC:(i + 1) * c.HC] for i in range(3))

    wq = wgt.tile([128, c.KD, c.HC], BF16, tag="wq")
    nc.gpsimd.dma_start(wq[:], rearr(io["wq"]))
    wk = wgt.tile([128, c.KD, c.HC], BF16, tag="wk")
    nc.gpsimd.dma_start(wk[:], rearr(io["wk"]))
    wv = wgt.tile([128, c.KD, c.HC], BF16, tag="wv")
    nc.gpsimd.dma_start(wv[:], rearr(io["wv"]))
    wo = wgt.tile([128, c.KD, c.D], BF16, tag="wo")
    nc.scalar.dma_start(wo[:], rearr(io["wo"]))
    wf1_0 = wgt.tile([128, c.KD, c.HCB], BF16, tag="wf1_0")
    nc.scalar.dma_start(wf1_0[:], rearr(io["wf1"])[:, :, 0:c.HCB])
    wf2r = io["wf2"].rearrange("(o p) d -> p o d", p=128)
    wf2_0 = wgt.tile([128, c.KHB, c.OCB], BF16, tag="wf2_0")
    nc.scalar.dma_start(wf2_0[:], wf2r[:, 0:c.KHB, 0:c.OCB])

    # own-chunk x (stats + oproj residual)
    xown = const.tile([128, c.KD, c.LCH], BF16, tag="xown", name="xown")
    nc.sync.dma_start(xown[:], rearr(io["xo"]))

    # ---------------- phase A: own-chunk LN1 stats + AllGather -------------
    ag_in = dram.tile([3, c.LCH], BF16, tag="ag_in", name="ag_in")
    ag_out = dram.tile([3 * c.NC, c.LCH], BF16, tag="ag_out", name="ag_out")

    inv_d = 1.0 / c.D
    with ExitStack() as ph0:
        ph0.enter_context(tc.high_priority())
        xsqp = ph0.enter_context(tc.tile_pool(name="xsqp", bufs=2))
        pmm = ph0.enter_context(tc.tile_pool(name="pmm0", bufs=2, space="PSUM"))
        ps1 = pmm.tile([128, c.LCH], F32, tag="mm", name="ps1")[0:1, :]
        for o in range(c.KD):
            mm(ps1, ones[:], xown[:, o, :], o == 0, o == c.KD - 1)
        ps2 = pmm.tile([128, c.LCH], F32, tag="mm", name="ps2")[0:1, :]
        for o in range(c.KD):
            xsq = xsqp.tile([128, c.LCH], BF16, tag="xsq", name="xsq")
            nc.vector.tensor_tensor(xsq[:], xown[:, o, :], xown[:, o, :], ALU.mult)
            mm(ps2, ones[:], xsq[:], o == 0, o == c.KD - 1)
        mu = small.tile([1, c.LCH], F32, tag="mu", name="mu")
        nc.vector.tensor_scalar_mul(mu, ps1, inv_d)
        ex2 = small.tile([1, c.LCH], F32, tag="ex2", name="ex2")
        nc.vector.tensor_scalar_mul(ex2, ps2, inv_d)
        var = small.tile([1, c.LCH], F32, tag="var", name="var")
        nc.vector.tensor_tensor(var, mu, mu, ALU.mult)
        nc.vector.tensor_tensor(var, ex2, var, ALU.subtract)
        std_f = small.tile([1, c.LCH], F32, tag="std_f", name="std_f")
        nc.scalar.activation(std_f, var, AF.Sqrt, bias=eps_t[:])
        A_t = small.tile([1, c.LCH], BF16, tag="A_t", name="A_t")
        with nc.allow_low_precision(reason="bf16 LN scale is intentional"):
            nc.vector.reciprocal(A_t, std_f)
        negmu_t = small.tile([1, c.LCH], BF16, tag="negmu_t", name="negmu_t")
        nc.vector.tensor_scalar_mul(negmu_t, mu, -1.0)
        std_t = small.tile([1, c.LCH], BF16, tag="std_t", name="std_t")
        nc.vector.tensor_copy(std_t[:], std_f[:])
        nc.scalar.dma_start(ag_in[0:1, :], negmu_t[:])
        nc.scalar.dma_start(ag_in[1:2, :], std_t[:])
        nc.scalar.dma_start(ag_in[2:3, :], A_t[:])
        nc.gpsimd.collective_compute(
            "AllGather", ALU.bypass, replica_groups=[list(range(c.NC))],
            ins=[ag_in[:].opt()], outs=[ag_out[:].opt()])
    ago3 = ag_out[:].rearrange("(ch s) t -> s ch t", s=3)

    isc = 1.0 / float(np.sqrt(c.HD))

    # a2a bounce buffers, one per 256-col piece, in separate dram pools
    a2a_in, a2a_out = [], []
    for p in range(2):
        din = ctx.enter_context(tc.tile_pool(name=f"da{p}", bufs=1, space="DRAM"))
        dout = ctx.enter_context(tc.tile_pool(name=f"db{p}", bufs=1, space="DRAM"))
        a2a_in.append(din.tile([c.NC * c.HC, c.SW], BF16,
                               tag=f"a2a_in{p}", name=f"a2a_in{p}"))
        a2a_out.append(dout.tile([c.NC * c.HC, c.SW], BF16,
                                 tag=f"a2a_out{p}", name=f"a2a_out{p}"))

    # persistent attention tiles
    kvqy = ctx.enter_context(tc.tile_pool(name="kvqy", bufs=1))
    kT = [kvqy.tile([128, c.T], BF16, tag=f"kT{b}", name=f"kT{b}")
          for b in range(c.B)]
    qT = [kvqy.tile([128, c.T], BF16, tag=f"qT{b}", name=f"qT{b}")
          for b in range(c.B)]
    v_sb = [[kvqy.tile([128, c.NHC * 65], BF16, tag=f"v{b}_{a}",
                       name=f"v{b}_{a}") for a in range(c.NKC)]
            for b in range(c.B)]
    for b in range(c.B):
        for a in range(c.NKC):
            nc.vector.tensor_copy(
                v_sb[b][a][:].rearrange("p (h e) -> p h e", e=65)[:, :, 64:65],
                ones_f[:, 0:1].to_broadcast((128, c.NHC, 1)))

    x2b = ctx.enter_context(tc.tile_pool(name="x2b", bufs=1)).tile(
        [128, c.KD, c.TQ], BF16, tag="x2b")
    x2n = ctx.enter_context(tc.tile_pool(name="x2n", bufs=1)).tile(
        [128, c.KD, c.TQ], BF16, tag="x2n")
    mu2 = small.tile([1, c.TQ], F32, tag="mu2", name="mu2")
    ex22 = small.tile([1, c.TQ], F32, tag="ex22", name="ex22")

    def quad(b, s, p, grp, pools):
        """256-wide strip s of batch b, both heads; a2a piece p."""
        sgrp_p, ssb_p, psy_p, rcp_p, rb_p, ystr_p = pools
        n = 2 * s + 2                     # key chunks
        qsl = slice(c.SW * s, c.SW * (s + 1))
        psy = [psy_p.tile([65, 256], F32, tag=f"psy{h}", name=f"psy{h}")
               for h in range(2)]
        sg = [None, None]
        for g0 in range(0, n, grp):
            g1 = min(g0 + grp, n)
            gw = (g1 - g0) * c.SW
            for h in range(2):
                sg[h] = sgrp_p.tile([128, grp * c.SW], F32, tag=f"s{h}",
                                    name=f"sg{h}")
            # interleave heads so row-tiled MM pairs overlap in the array
            for a in range(g0, g1):
                r = (a - g0) * c.SW
                for h in range(2):
                    rs = slice(64 * h, 64 * h + 64)
                    mm(sg[h][:, r:r + c.SW],
                       kT[b][rs, a * 128:(a + 1) * 128],
                       qT[b][rs, qsl], True, True)
            for h in range(2):
                ssb = ssb_p.tile([128, grp * c.SW], BF16, tag=f"e{h}",
                                 name=f"ssb{h}")
                nc.scalar.activation(ssb[:, 0:gw], sg[h][:, 0:gw], AF.Exp,
                                     scale=isc)
                for a in (2 * s, 2 * s + 1):   # diagonal chunks
                    if g0 <= a < g1:
                        r = (a - g0) * c.SW
                        nc.vector.tensor_tensor(
                            ssb[:, r:r + c.SW], ssb[:, r:r + c.SW],
                            (M0 if a == 2 * s else M1)[:], ALU.mult)
                for a in range(g0, g1):
                    r = (a - g0) * c.SW
                    mm(psy[h][:], v_sb[b][a][:, h * 65:h * 65 + 65],
                       ssb[:, r:r + c.SW], a == 0, a == n - 1)
        rcp = rcp_p.tile([1, 512], BF16, tag="rcp", name="rcp")
        with nc.allow_low_precision(reason="bf16 softmax denom is intentional"):
            for h in range(2):
                nc.vector.reciprocal(rcp[:, 256 * h:256 * h + 256],
                                     psy[h][64:65, :])
        rb = rb_p.tile([64, 512], F32, tag="rb", name="rb")
        mm(rb[:], ones_c[:, 0:64], rcp[:], True, True)
        rbs = rcp_p.tile([64, 512], BF16, tag="rbs", name="rbs")
        nc.vector.tensor_copy(rbs[:], rb[:])
        ystr = ystr_p.tile([128, c.SW], BF16, tag="ystr", name="ystr")
        for h in range(2):
            nc.vector.tensor_tensor(
                ystr[64 * h:64 * h + 64, :],
                psy[h][0:64, :],
                rbs[:, 256 * h:256 * h + 256], ALU.mult)
        j = b * c.TP + s // 2
        nc.sync.dma_start(a2a_in[p][c.HC * j:c.HC * (j + 1), :], ystr[:])

    def a2a(p):
        nc.gpsimd.collective_compute(
            "AllToAll", ALU.bypass, replica_groups=[list(range(c.NC))],
            ins=[a2a_in[p][:].opt()], outs=[a2a_out[p][:].opt()])

    xT = io["xT"]  # [D, B*T] bf16
    # fc1 weights: blocks 0-2 resident (block 0 preloaded above), 3-7 rotate
    wf1_res = {0: wf1_0}
    for hcb in (1, 2):
        wt = wgt.tile([128, c.KD, c.HCB], BF16, tag=f"wf1_{hcb}")
        wf1_res[hcb] = wt
    w1p = ctx.enter_context(tc.tile_pool(name="w1p", bufs=3))
    w2p = ctx.enter_context(tc.tile_pool(name="w2p", bufs=3))

    def wf1_dma(tile_, hcb):
        nc.scalar.dma_start(
            tile_[:], rearr(io["wf1"])[:, :, hcb * c.HCB:(hcb + 1) * c.HCB])

    # ------- phase C: QKV chunks + eager strip-0 attention ----------------
    with ExitStack() as ph1:
        xcp = ph1.enter_context(tc.tile_pool(name="xcp", bufs=2))
        nmp = ph1.enter_context(tc.tile_pool(name="nmp", bufs=2))
        vtp = ph1.enter_context(tc.tile_pool(name="vtp", bufs=2))
        abp = ph1.enter_context(tc.tile_pool(name="abp", bufs=1, space="PSUM"))
        absb = ph1.enter_context(tc.tile_pool(name="absb", bufs=2))
        pmm = ph1.enter_context(tc.tile_pool(name="pmm1", bufs=1, space="PSUM"))
        ptr = ph1.enter_context(tc.tile_pool(name="ptr", bufs=1, space="PSUM"))
        sgrpE = ph1.enter_context(tc.tile_pool(name="sgrpE", bufs=1, space="PSUM"))
        psyE = ph1.enter_context(tc.tile_pool(name="psyE", bufs=1, space="PSUM"))
        rbE = ph1.enter_context(tc.tile_pool(name="rbE", bufs=1, space="PSUM"))
        ssbE = ph1.enter_context(tc.tile_pool(name="ssbE", bufs=4))
        rcpE = ph1.enter_context(tc.tile_pool(name="rcpE", bufs=2))
        ystrE = ph1.enter_context(tc.tile_pool(name="ystrE", bufs=4))
        poolsE = (sgrpE, ssbE, psyE, rcpE, rbE, ystrE)
        for ch in range(c.NCH):
            b, t = ch // c.NTL, ch % c.NTL
            tsl = slice(ch * c.LCH, (ch + 1) * c.LCH)
            lsl = slice(t * c.LCH, (t + 1) * c.LCH)
            xc = xcp.tile([128, c.KD, c.LCH], BF16, tag="xc")
            nc.sync.dma_start(xc[:], rearr(xT)[:, :, tsl])
            # per-chunk AllGathered stats: [-mu; std] rows + A row
            nm_std = nmp.tile([2, c.LCH], BF16, tag="nm", name="nm_std")
            nc.scalar.dma_start(nm_std[:], ago3[0:2, ch, :])
            A_row = nmp.tile([1, c.LCH], BF16, tag="Ar", name="A_row")
            nc.scalar.dma_start(A_row[:], ago3[2:3, ch, :])
            # A broadcast: PE outer product -> bf16 SBUF copy
            abps = abp.tile([128, c.LCH], F32, tag="abps", name="abps")
            mm(abps[:], ones_c[:], A_row[:], True, True)
            Ab = absb.tile([128, c.LCH], BF16, tag="Ab", name="Ab")
            nc.vector.tensor_copy(Ab[:], abps[:])

            for (wt, pvx, dst) in ((wq, pvq, qT), (wk, pvk, kT)):
                ps = pmm.tile([128, c.LCH], F32, tag="mm", name="psqk")
                for o in range(c.KD):
                    mm(ps, wt[:, o, :], xc[:, o, :], o == 0, False)
                mm(ps, pvx, nm_std, False, True)
                nc.vector.tensor_tensor(dst[b][:, lsl], ps, Ab[:], ALU.mult)

            # v: feature-major then per-head DMA transpose into v_sb
            ps = pmm.tile([128, c.LCH], F32, tag="mm", name="psv")
            for o in range(c.KD):
                mm(ps, wv[:, o, :], xc[:, o, :], o == 0, False)
            mm(ps, pvv, nm_std, False, True)
            vT = vtp.tile([128, c.LCH], BF16, tag="vT", name="vT")
            nc.vector.tensor_tensor(vT[:], ps, Ab[:], ALU.mult)
            for s2 in range(c.LCH // 128):
                a = t * (c.LCH // 128) + s2
                pst = ptr.tile([128, 128], BF16, tag="tr", name="pst")
                nc.tensor.transpose(pst[:], vT[:, s2 * 128:(s2 + 1) * 128],
                                    ident[:])
                v3v = v_sb[b][a][:].rearrange("p (h e) -> p h e", e=65)[:, :, 0:64]
                p3v = pst[:].rearrange("p (h e) -> p h e", e=64)
                nc.vector.tensor_copy(v3v, p3v)

            # eager strip-0 for this chunk's block (piece 0)
            quad(b, 2 * t, 0, 2, poolsE)
            # resident fc1 weight blocks stream between exp bursts
            if ch == 3:
                wf1_dma(wf1_res[1], 1)
            if ch == 6:
                wf1_dma(wf1_res[2], 2)
    a2a(0)

    # ------- phase D: strip-1 quads ----------------------------------------
    with ExitStack() as ph2:
        sgrpP = ph2.enter_context(tc.tile_pool(name="sgrpP", bufs=1, space="PSUM"))
        psyP = ph2.enter_context(tc.tile_pool(name="psyP", bufs=1, space="PSUM"))
        rbP = ph2.enter_context(tc.tile_pool(name="rbP", bufs=2, space="PSUM"))
        ssbP = ph2.enter_context(tc.tile_pool(name="ssbP", bufs=3))
        rcpP = ph2.enter_context(tc.tile_pool(name="rcpP", bufs=2))
        ystrP = ph2.enter_context(tc.tile_pool(name="ystrP", bufs=4))
        poolsP = (sgrpP, ssbP, psyP, rcpP, rbP, ystrP)
        for b in range(c.B):
            for qb in range(c.QB):
                quad(b, 2 * qb + 1, 1, 4, poolsP)
    a2a(1)
    # wf2 streams on the Pool queue once collectives are done (rotating pool,
    # emitted in fc2 consumption order: dcb outer, khb inner)
    wf2_t = {}
    for dcb in range(c.D // c.OCB):
        for khb in range(c.FD // c.KHB):
            if khb == 0 and dcb == 0:
                wf2_t[(0, 0)] = wf2_0
                continue
            wt2 = w2p.tile([128, c.KHB, c.OCB], BF16, tag="wf2s", name="wt2")
            nc.gpsimd.dma_start(
                wt2[:], wf2r[:, khb * c.KHB:(khb + 1) * c.KHB,
                             dcb * c.OCB:(dcb + 1) * c.OCB])
            wf2_t[(khb, dcb)] = wt2

    # ------- phase E: per-piece oproj -> LN2 -> fc1; then fc2 --------------
    with ExitStack() as ph3:
        pmm = ph3.enter_context(tc.tile_pool(name="pmm2", bufs=2, space="PSUM"))
        bc2 = ph3.enter_context(tc.tile_pool(name="bc2", bufs=1, space="PSUM"))
        xsq2_p = ph3.enter_context(tc.tile_pool(name="xsq2", bufs=2))
        yfull_p = ph3.enter_context(tc.tile_pool(name="yfull", bufs=1))
        bcsb = ph3.enter_context(tc.tile_pool(name="bcsb", bufs=4))
        pmf = ph3.enter_context(tc.tile_pool(name="pmf", bufs=2, space="PSUM"))
        hsb_p = ph3.enter_context(tc.tile_pool(name="hsb", bufs=1))
        h_sb = hsb_p.tile([128, c.FD, c.TQ], BF16, tag="h")

        def piece_head(p):
            """oproj + LN2 stats/chain/apply for 256-col piece p."""
            csl = slice(c.SW * p, c.SW * (p + 1))
            yfull = yfull_p.tile([128, c.KD, c.SW], BF16, tag="yfull",
                                 name="yfull")
            nc.sync.dma_start(yfull[:], rearr(a2a_out[p][:]))
            for oc in range(c.KD):
                ps = pmm.tile([128, c.SW], F32, tag="mm", name="pso")
                for k in range(c.KD):
                    mm(ps, wo[:, k, oc * 128:(oc + 1) * 128],
                       yfull[:, k, :], k == 0, k == c.KD - 1)
                nc.vector.scalar_tensor_tensor(
                    x2b[:, oc, csl], ps, bo[:, oc:oc + 1],
                    xown[:, oc, csl], ALU.add, ALU.add)
            # LN2 stats for this piece's tokens
            ps1 = pmm.tile([128, c.SW], F32, tag="mm", name="l2a")[0:1, :]
            for o in range(c.KD):
                mm(ps1, ones[:], x2b[:, o, csl], o == 0, o == c.KD - 1)
            ps2 = pmm.tile([128, c.SW], F32, tag="mm", name="l2b")[0:1, :]
            for o in range(c.KD):
                xsq2 = xsq2_p.tile([128, c.SW], BF16, tag="xsq2", name="xsq2")
                nc.vector.tensor_tensor(xsq2, x2b[:, o, csl],
                                        x2b[:, o, csl], ALU.mult)
                mm(ps2, ones[:], xsq2, o == 0, o == c.KD - 1)
            mu2p = mu2[:, csl]
            ex2p = ex22[:, csl]
            nc.vector.tensor_scalar_mul(mu2p, ps1, inv_d)
            nc.vector.tensor_scalar_mul(ex2p, ps2, inv_d)
            var2 = small.tile([1, c.SW], F32, tag=f"var2_{p}", name=f"var2_{p}")
            nc.vector.tensor_tensor(var2, mu2p, mu2p, ALU.mult)
            nc.vector.tensor_tensor(var2, ex2p, var2, ALU.subtract)
            std2 = small.tile([1, c.SW], F32, tag=f"std2_{p}", name=f"std2_{p}")
            nc.scalar.activation(std2, var2, AF.Sqrt, bias=eps_t[:])
            A2_ = small.tile([1, c.SW], BF16, tag=f"A2_{p}", name=f"A2_{p}")
            with nc.allow_low_precision(reason="bf16 LN scale is intentional"):
                nc.vector.reciprocal(A2_, std2)
            B2_ = small.tile([1, c.SW], BF16, tag=f"B2_{p}", name=f"B2_{p}")
            nc.vector.scalar_tensor_tensor(B2_, mu2p, -1.0, A2_, ALU.mult,
                                           ALU.mult)
            a2ps = bc2.tile([128, c.SW], F32, tag="a2ps", name="a2ps")
            mm(a2ps[:], ones_c[:], A2_[:], True, True)
            A2b = bcsb.tile([128, c.SW], BF16, tag="A2b", name="A2b")
            nc.vector.tensor_copy(A2b[:], a2ps[:])
            b2ps = bc2.tile([128, c.SW], F32, tag="b2ps", name="b2ps")
            mm(b2ps[:], ones_c[:], B2_[:], True, True)
            B2b = bcsb.tile([128, c.SW], BF16, tag="B2b", name="B2b")
            nc.vector.tensor_copy(B2b[:], b2ps[:])
            for o in range(c.KD):
                nc.vector.tensor_tensor(x2n[:, o, csl], x2b[:, o, csl],
                                        A2b[:], ALU.mult)
                nc.vector.tensor_tensor(x2n[:, o, csl], x2n[:, o, csl],
                                        B2b[:], ALU.add)

        def fc1_block(p, hcb, wt):
            csl = slice(c.SW * p, c.SW * (p + 1))
            for j in range(c.HCB // 128):
                hidx = hcb * (c.HCB // 128) + j
                ps = pmf.tile([128, c.SW], F32, tag="mmf", name="psf")
                for o in range(c.KD):
                    mm(ps, wt[:, o, j * 128:(j + 1) * 128],
                       x2n[:, o, csl], o == 0, o == c.KD - 1)
                nc.scalar.activation(h_sb[:, hidx, csl], ps,
                                     AF.Gelu_apprx_tanh,
                                     bias=bf1[:, hidx:hidx + 1])

        piece_head(0)
        for hcb in range(3):                       # resident blocks, piece 0
            fc1_block(0, hcb, wf1_res[hcb])
        piece_head(1)
        w1rot = {}
        for hcb in (3, 4):                          # prefetch rotation ahead
            w1t = w1p.tile([128, c.KD, c.HCB], BF16, tag="wf1s", name="w1t")
            w1rot[hcb] = w1t
            wf1_dma(w1t, hcb)
        for hcb in range(3):                        # resident blocks, piece 1
            fc1_block(1, hcb, wf1_res[hcb])
        nhcb = c.DFF // c.HCB
        for hcb in range(3, nhcb):
            wt = w1rot.pop(hcb)
            fc1_block(0, hcb, wt)
            fc1_block(1, hcb, wt)
            if hcb + 2 < nhcb:
                w1t2 = w1p.tile([128, c.KD, c.HCB], BF16, tag="wf1s", name="w1t2")
                w1rot[hcb + 2] = w1t2
                wf1_dma(w1t2, hcb + 2)

        # fc2 + residual -> out
        outp = ph3.enter_context(tc.tile_pool(name="outp", bufs=2))
        pfc2 = ph3.enter_context(tc.tile_pool(name="pfc2", bufs=1, space="PSUM"))
        for dcb in range(c.D // c.OCB):
            nb = c.OCB // 128
            psums = [pfc2.tile([128, c.TQ], F32, tag=f"fc2_{i}", name=f"fc2_{i}")
                     for i in range(nb)]
            for khb in range(c.FD // c.KHB):
                wt2 = wf2_t[(khb, dcb)]
                for k2 in range(c.KHB):
                    kh = khb * c.KHB + k2
                    for j in range(nb):
                        mm(psums[j], wt2[:, k2, j * 128:(j + 1) * 128],
                           h_sb[:, kh, :], kh == 0, kh == c.FD - 1)
            for j in range(nb):
                o = dcb * nb + j
                ot = outp.tile([128, c.TQ], F32, tag="ot", name="ot")
                nc.vector.scalar_tensor_tensor(ot[:], psums[j], bf2[:, o:o + 1],
                                               x2b[:, o, :], ALU.add, ALU.add)
                nc.sync.dma_start(rearr(io["out"])[:, o, :], ot[:])


# ---------------- host-side sharding ----------------

def pack_pf(v, D):
    """[D] per-feature vector -> [128, D//128] with [p, o] = v[128*o + p]."""
    return np.ascontiguousarray(np.asarray(v, np.float32).reshape(D // 128, 128).T)


def make_in_maps(inputs, cfg):
    import ml_dtypes
    bf = ml_dtypes.bfloat16
    c = cfg
    x = np.asarray(inputs["x"], np.float32)
    w_qkv = np.asarray(inputs["w_qkv"], np.float32)
    b_qkv = np.asarray(inputs["b_qkv"], np.float32)
    w_o = np.ascontiguousarray(np.asarray(inputs["w_o"], np.float32))
    w_fc1 = np.ascontiguousarray(np.asarray(inputs["w_fc1"], np.float32))
    w_fc2 = np.ascontiguousarray(np.asarray(inputs["w_fc2"], np.float32))
    D = c.D

    xT_all = np.concatenate([x[b].T for b in range(c.B)], axis=1)
    xT_all = np.ascontiguousarray(xT_all).astype(bf)  # [D, B*T]

    # fold LN affine into projection weights
    g1 = np.asarray(inputs["ln1_g"], np.float32)
    b1 = np.asarray(inputs["ln1_b"], np.float32)
    g2 = np.asarray(inputs["ln2_g"], np.float32)
    b2 = np.asarray(inputs["ln2_b"], np.float32)
    w_qkv_f = w_qkv * g1[:, None]
    b_qkv_f = b_qkv + b1 @ w_qkv
    w_fc1_f = np.ascontiguousarray((w_fc1 * g2[:, None])).astype(bf)
    b_fc1_f = np.asarray(inputs["b_fc1"], np.float32) + b2 @ w_fc1

    in_maps = []
    for core in range(c.NC):
        hc0 = core * c.HC
        qs, ks, vs = hc0, D + hc0, 2 * D + hc0
        pv2 = np.stack([
            np.concatenate([w_qkv_f[:, qs:qs + c.HC].sum(0),
                            w_qkv_f[:, ks:ks + c.HC].sum(0),
                            w_qkv_f[:, vs:vs + c.HC].sum(0)]),
            np.concatenate([b_qkv_f[qs:qs + c.HC],
                            b_qkv_f[ks:ks + c.HC],
                            b_qkv_f[vs:vs + c.HC]]),
        ]).astype(bf)
        m = {
            "xT": xT_all,
            "xo": np.ascontiguousarray(
                np.asarray(xT_all)[:, core * c.LCH:(core + 1) * c.LCH]),
            "wq": np.ascontiguousarray(w_qkv_f[:, qs:qs + c.HC]).astype(bf),
            "wk": np.ascontiguousarray(w_qkv_f[:, ks:ks + c.HC]).astype(bf),
            "wv": np.ascontiguousarray(w_qkv_f[:, vs:vs + c.HC]).astype(bf),
            "pv2": pv2,
            "wo": w_o.astype(bf),
            "pm": np.concatenate([
                pack_pf(inputs["b_o"], D),
                pack_pf(b_fc1_f, c.DFF),
                pack_pf(inputs["b_fc2"], D),
            ], axis=1).astype(np.float32),
            "wf1": w_fc1_f,
            "wf2": w_fc2.astype(bf),
        }
        in_maps.append(m)
    return in_maps


def assemble_output(results, cfg):
    c = cfg
    out = np.empty((c.B, c.T, c.D), np.float32)
    for core in range(c.NC):
        b, p = core // c.TP, core % c.TP
        out[b, p * c.TQ:(p + 1) * c.TQ, :] = results[core]["out"].T
    return out


def build_nc(cfg, reps=1):
    nc = bacc.Bacc("TRN2", target_bir_lowering=False, debug=False,
                   num_devices=cfg.NC, name="nn_block")
    c = cfg
    io = {}
    specs = {
        "xT": ((c.D, c.B * c.T), BF16),
        "xo": ((c.D, c.LCH), BF16),
        "wq": ((c.D, c.HC), BF16),
        "wk": ((c.D, c.HC), BF16),
        "wv": ((c.D, c.HC), BF16),
        "pv2": ((2, 3 * c.HC), BF16),
        "pm": ((128, 2 * c.KD + c.FD), F32),
        "wo": ((c.D, c.D), BF16),
        "wf1": ((c.D, c.DFF), BF16),
        "wf2": ((c.DFF, c.D), BF16),
    }
    for name, (shape, dt) in specs.items():
        io[name] = nc.declare_dram_parameter(name, list(shape), dt,
                                             isOutput=False).ap()
    io["out"] = nc.declare_dram_parameter("out", [c.D, c.TQ], F32,
                                          isOutput=True).ap()
    with tile.TileContext(nc) as tc:
        for _ in range(reps):
            with ExitStack() as ctx:
                emit(ctx, tc, io, cfg)
    nc.compile()
    return nc


_CACHE = {}


def kernel(**inputs) -> np.ndarray:
    from concourse.bass_utils import run_bass_kernel_spmd
    cfg = CFG
    if "nc" not in _CACHE:
        _CACHE["nc"] = build_nc(cfg)
    nc = _CACHE["nc"]
    in_maps = make_in_maps(inputs, cfg)
    res = run_bass_kernel_spmd(nc, in_maps, core_ids=list(range(cfg.NC)))
    return assemble_output(res.results, cfg)
